# revision 30
# baseline (speedup 1.0000x reference)
"""Trainium2 Bass kernel for nn_Attention_42202348650800.

Full causal attention block: fused QKV projection + RoPE + causal softmax
attention + output projection.  B=2, T=2048, HIDDEN=1024, H=HKV=16, D=64.

Sharding (8 NeuronCores): data-parallel over batch (2) x tensor-parallel over
heads (4 groups of 4 heads).  core c -> batch b = c // 4, head group g = c % 4
(heads 4g..4g+3).  Each core computes a partial output projection
y_partial^T = w_o[:, jslice].T @ ctx^T  in [1024, 2048]; the host sums the 4
partials of each batch.

Device-side layout notes:
 - All matmul operands are float32r (fp32 with 11-bit mantissa; inputs
   pre-rounded on host).  PSUM accumulation is fp32.
 - Q^T/K^T are built directly in [d, t] layout (transposed), with the head-dim
   d PERMUTED per head as d' = (2i -> i, 2i+1 -> 32+i) so that RoPE's
   rotate-half becomes an even/odd partition swap (DVE stream_shuffle).
   cos/sin tables are pre-permuted and sign-folded on the host.
 - Scores are computed transposed (S^T[s, t]) so softmax's sum runs over the
   matmul contraction: ctx^T accumulates  V'^T @ expS^T  where V' = [V | 1];
   the ones column makes the softmax denominator fall out as row 64 of the
   ctx^T PSUM tile.
 - Causal mask is applied after exp by GPSIMD affine_select (fill 0).
"""

import math
import os
import sys

import numpy as np

sys.path.insert(0, "/opt/trn_rl_repo")

from contextlib import ExitStack

import concourse.bass as bass
import concourse.tile as tile
import concourse.mybir as mybir
from concourse import bacc, bass_utils

# Problem constants
B, T, HID = 2, 2048, 1024
H, D = 16, 64
NCORES = 8
HPC = 4          # heads per core
NPAIR = 2        # head pairs per core
KT = HID // 128  # 8 contraction tiles for qkv
F32 = mybir.dt.float32
F32R = mybir.dt.float32r
BF16 = mybir.dt.bfloat16
DT_E = BF16      # dtype of exp(scores) and V' (ctx matmul operands)
DT_QK = BF16     # dtype of Q^T/K^T tiles (scores matmul operands)
DT_O = BF16      # dtype of normalized ctx + w_o (output-proj operands)
ONES_NP = mybir.dt.np(DT_E)

SCALE = 1.0 / math.sqrt(D)

_NC_CACHE = {}


def round_fp32r(x: np.ndarray) -> np.ndarray:
    """Round fp32 to float32r grid (11-bit mantissa, RNE-ish)."""
    u = np.ascontiguousarray(x, dtype=np.float32).view(np.uint32).astype(np.uint64)
    u = (u + 0x7FF + ((u >> 12) & 1)) & ~np.uint64(0xFFF)
    return u.astype(np.uint32).view(np.float32)


def build_nc():
    """Build + compile the per-core Bass program (identical SPMD on all cores)."""
    if "nc" in _NC_CACHE:
        return _NC_CACHE["nc"]

    nc = bacc.Bacc("TRN2", target_bir_lowering=False, debug=False)

    # DRAM I/O (per core)
    hx_d = nc.dram_tensor("hx", [128, KT * T], F32R, kind="ExternalInput").ap()
    wq_d = nc.dram_tensor("wq", [128, KT * 256], F32R, kind="ExternalInput").ap()
    wk_d = nc.dram_tensor("wk", [128, KT * 256], F32R, kind="ExternalInput").ap()
    wv_d = nc.dram_tensor("wv", [128, KT * 256], F32R, kind="ExternalInput").ap()
    wo_d = nc.dram_tensor("wo", [128, 2 * HID], DT_O, kind="ExternalInput").ap()
    cos_d = nc.dram_tensor("cosx", [128, T], F32, kind="ExternalInput").ap()
    sin_d = nc.dram_tensor("sinx", [128, T], F32, kind="ExternalInput").ap()
    ones_d = nc.dram_tensor("ones", [128, 64], DT_E, kind="ExternalInput").ap()
    yt_d = nc.dram_tensor("yt", [HID, T], F32, kind="ExternalOutput").ap()

    Exp = mybir.ActivationFunctionType.Exp
    is_ge = mybir.AluOpType.is_ge
    SWAP_MASK = [i ^ 1 for i in range(32)]

    with tile.TileContext(nc) as tc:
        with ExitStack() as octx:
            # ---- persistent pools (live for the whole kernel) ----
            qk_pool = octx.enter_context(tc.tile_pool(name="qk", bufs=1))
            vp_pool = octx.enter_context(tc.tile_pool(name="vp", bufs=1))

            QT = [qk_pool.tile([128, T], DT_QK, name=f"QT{p}") for p in range(NPAIR)]
            KTt = [qk_pool.tile([128, T], DT_QK, name=f"KTt{p}") for p in range(NPAIR)]
            Vp = vp_pool.tile([128, 16 * 260], DT_E, name="Vp")

            # ================= stage 1: QKV projection + RoPE + V' =========
            with ExitStack() as s1:
                in_pool = s1.enter_context(tc.tile_pool(name="inp", bufs=1))
                rp_pool = s1.enter_context(tc.tile_pool(name="ropep", bufs=1))
                ps1 = s1.enter_context(tc.tile_pool(name="ps1", bufs=1, space="PSUM"))

                wq_t = in_pool.tile([128, KT * 256], F32R, name="wq_t")
                wk_t = in_pool.tile([128, KT * 256], F32R, name="wk_t")
                cos_t = in_pool.tile([128, T], F32, name="cos_t")
                sin_t = in_pool.tile([128, T], F32, name="sin_t")
                hx_t = in_pool.tile([128, KT * T], F32R, name="hx_t")
                wv_t = in_pool.tile([128, KT * 256], F32R, name="wv_t")

                nc.sync.dma_start(wq_t[:], wq_d[:])
                nc.sync.dma_start(wk_t[:], wk_d[:])
                for k in range(KT):
                    nc.sync.dma_start(
                        hx_t[:, k * T:(k + 1) * T], hx_d[:, k * T:(k + 1) * T]
                    )
                nc.sync.dma_start(wv_t[:], wv_d[:])
                nc.sync.dma_start(cos_t[:], cos_d[:])
                nc.sync.dma_start(sin_t[:], sin_d[:])

                # ones columns of V' (softmax-denominator trick), DMA'd into
                # every 65th column
                vp_ones = Vp[:].rearrange("p (s h c) -> p s h c", s=16, h=4)
                nc.sync.dma_start(
                    vp_ones[:, :, :, 64:65],
                    ones_d[:].rearrange("p (s h c) -> p s h c", s=16, h=4),
                )

                # Q^T and K^T (pair-packed [2*64d, T]) + RoPE
                for pr in range(NPAIR):
                    for which, w_t, OUT in (("q", wq_t, QT), ("k", wk_t, KTt)):
                        raw = rp_pool.tile([128, T], F32, name=f"raw_{pr}{which}",
                                           tag="raw", bufs=2)
                        # k-outer: the first matmul only needs hx k-tile 0, so
                        # compute starts as soon as the first DMAs land
                        psqs = [
                            ps1.tile([128, 512], F32, name=f"psqk{pr}{which}{n}",
                                     tag="psqk", bufs=8)
                            for n in range(4)
                        ]
                        for k in range(KT):
                            for n in range(4):
                                nc.tensor.matmul(
                                    psqs[n][:],
                                    w_t[:, k * 256 + pr * 128: k * 256 + pr * 128 + 128],
                                    hx_t[:, k * T + n * 512: k * T + (n + 1) * 512],
                                    start=(k == 0), stop=(k == KT - 1),
                                )
                        for n in range(4):
                            nc.vector.tensor_copy(raw[:, n * 512:(n + 1) * 512],
                                                  psqs[n][:])
                        rot = rp_pool.tile([128, T], F32, name=f"rot_{pr}{which}",
                                           tag="rot")
                        nc.vector.stream_shuffle(rot[:], raw[:], SWAP_MASK)
                        tmp2 = rp_pool.tile([128, T], F32, name=f"ts_{pr}{which}",
                                            tag="tmp2")
                        nc.gpsimd.tensor_mul(tmp2[:], rot[:], sin_t[:])
                        # tmp1 reuses rot's slot (rot is dead once tmp2 is made)
                        tmp1 = rp_pool.tile([128, T], F32, name=f"tc_{pr}{which}",
                                            tag="rot")
                        nc.vector.tensor_mul(tmp1[:], raw[:], cos_t[:])
                        nc.vector.tensor_add(OUT[pr][:], tmp1[:], tmp2[:])

                # V' tiles: [V_h | 1] interleaved, natural d order
                for st in range(16):
                    psv = ps1.tile([128, 256], F32, name=f"psv{st}", tag="psqk",
                                   bufs=8)
                    for k in range(KT):
                        nc.tensor.matmul(
                            psv[:],
                            hx_t[:, k * T + st * 128: k * T + st * 128 + 128],
                            wv_t[:, k * 256:(k + 1) * 256],
                            start=(k == 0), stop=(k == KT - 1),
                        )
                    dst = Vp[:, st * 260:(st + 1) * 260]
                    dst = dst.rearrange("p (h c) -> p h c", h=4)[:, :, 0:64]
                    src = psv[:].rearrange("p (h c) -> p h c", h=4)
                    nc.vector.tensor_copy(dst, src)

            # late persistent pools (allocated after stage-1 pools release,
            # so they reuse stage-1's SBUF space)
            wo_pool = octx.enter_context(tc.tile_pool(name="wop", bufs=1))
            ctx_pool = octx.enter_context(tc.tile_pool(name="ctxp", bufs=1))
            dn_pool = octx.enter_context(tc.tile_pool(name="dnp", bufs=1))
            wo_t = wo_pool.tile([128, 2 * HID], DT_O, name="wo_t")
            nc.sync.dma_start(wo_t[:], wo_d[:])
            CTXU = [ctx_pool.tile([128, T], DT_O, name=f"CTXU{p}") for p in range(NPAIR)]
            # one denominator row + reciprocal row per (pair, head, t-block):
            # compute-engine SBUF APs must start at partition 0/32/64/96, so
            # each row lives in its own [1, 512] tile.
            Dn = [dn_pool.tile([1, 512], F32, name=f"Dn{r}") for r in range(16)]
            Rn = [dn_pool.tile([1, 512], F32, name=f"Rn{r}") for r in range(16)]

            # ================= stage 2: attention ==========================
            with ExitStack() as s2:
                e_pool = s2.enter_context(tc.tile_pool(name="ep", bufs=1))
                ps_s = s2.enter_context(tc.tile_pool(name="pss", bufs=1, space="PSUM"))
                ps_c = s2.enter_context(tc.tile_pool(name="psc", bufs=1, space="PSUM"))

                NPIPE = 3  # software-pipeline depth: ctx matmuls trail the
                #            scores/exp stream so the PE FIFO never head-blocks
                for pr in range(NPAIR):
                    hA, hB = 2 * pr, 2 * pr + 1
                    for tb in range(4):
                        ctxP = [
                            ps_c.tile([65, 512], F32, name=f"ctx{pr}{tb}{ab}",
                                      tag="ctx", bufs=3)
                            for ab in range(2)
                        ]
                        shi = 4 * (tb + 1)
                        pend = []

                        def flush_ctx(pending):
                            si, col0, tiles = pending
                            for ab, hh in ((0, hA), (1, hB)):
                                nc.tensor.matmul(
                                    ctxP[ab][:, col0:],
                                    Vp[:, si * 260 + hh * 65: si * 260 + hh * 65 + 65],
                                    tiles[ab][:, col0:],
                                    start=(si == 0), stop=(si == shi - 1),
                                )

                        for si in range(shi):
                            j = si - 4 * tb
                            col0 = 0 if j < 0 else (0, 128, 256, 256)[j]
                            base = tb * 512 + col0 - si * 128
                            ncol = 512 - col0
                            etiles = []
                            for ab, hh in ((0, hA), (1, hB)):
                                pp = slice(64 * ab, 64 * ab + 64)
                                pss = ps_s.tile([128, 512], F32,
                                                name=f"s{pr}{tb}{si}{ab}",
                                                tag="s", bufs=5)
                                nc.tensor.matmul(
                                    pss[:, col0:],
                                    KTt[pr][pp, si * 128:(si + 1) * 128],
                                    QT[pr][pp, tb * 512 + col0:(tb + 1) * 512],
                                    start=True, stop=True,
                                )
                                ee = e_pool.tile([128, 512], DT_E,
                                                 name=f"e{pr}{tb}{si}{ab}",
                                                 tag="e", bufs=12)
                                nc.scalar.activation(ee[:, col0:], pss[:, col0:],
                                                     Exp, scale=SCALE)
                                if j >= 0:
                                    nc.gpsimd.affine_select(
                                        ee[:, col0:], ee[:, col0:],
                                        pattern=[[1, ncol]], compare_op=is_ge,
                                        fill=0.0, base=base,
                                        channel_multiplier=-1,
                                    )
                                etiles.append(ee)
                            pend.append((si, col0, etiles))
                            if len(pend) > NPIPE:
                                flush_ctx(pend.pop(0))
                        while pend:
                            flush_ctx(pend.pop(0))
                        # evacuate ctx + denominators
                        for ab, hh in ((0, hA), (1, hB)):
                            r = pr * 8 + ab * 4 + tb
                            nc.vector.tensor_copy(
                                CTXU[pr][64 * ab:64 * ab + 64, tb * 512:(tb + 1) * 512],
                                ctxP[ab][0:64, :],
                            )
                            nc.scalar.copy(Dn[r][:], ctxP[ab][64:65, :])
                    # normalize this pair as soon as its denominators are done
                    for ab in range(2):
                        for tb in range(4):
                            r = pr * 8 + ab * 4 + tb
                            nc.vector.reciprocal_approx_fast(Rn[r][:], Dn[r][:])
                            # SBUF tensor_tensor requires equal base partitions,
                            # so broadcast to all 128 and slice the right half.
                            rb = e_pool.tile([128, 512], F32, name=f"rb{pr}{ab}{tb}",
                                             tag="rb", bufs=2)
                            nc.gpsimd.partition_broadcast(rb[:], Rn[r][:])
                            sl = slice(64 * ab, 64 * ab + 64)
                            cs = slice(tb * 512, (tb + 1) * 512)
                            nc.vector.tensor_mul(CTXU[pr][sl, cs], CTXU[pr][sl, cs],
                                                 rb[sl, :])

            # ================= stage 3: output projection ==================
            with ExitStack() as s3:
                y_pool = s3.enter_context(tc.tile_pool(name="yp", bufs=1))
                ps_y = s3.enter_context(tc.tile_pool(name="psy", bufs=1, space="PSUM"))
                for m in range(8):
                    yst = y_pool.tile([128, T], F32, name=f"yst{m}", tag="yst",
                                      bufs=2)
                    for n in range(4):
                        psy = ps_y.tile([128, 512], F32, name=f"psy{m}{n}", tag="psy",
                                        bufs=4)
                        for jt in range(2):
                            nc.tensor.matmul(
                                psy[:],
                                wo_t[:, jt * HID + m * 128: jt * HID + (m + 1) * 128],
                                CTXU[jt][:, n * 512:(n + 1) * 512],
                                start=(jt == 0), stop=(jt == 1),
                            )
                        if n % 2 == 0:
                            nc.vector.tensor_copy(yst[:, n * 512:(n + 1) * 512], psy[:])
                        else:
                            nc.scalar.copy(yst[:, n * 512:(n + 1) * 512], psy[:])
                    nc.sync.dma_start(yt_d[m * 128:(m + 1) * 128, :], yst[:])

    nc.compile()
    _NC_CACHE["nc"] = nc
    return nc


# RoPE head-dim permutation: d' = 2i -> i, 2i+1 -> 32+i
_PERM = np.empty(64, dtype=np.int64)
_PERM[0::2] = np.arange(32)
_PERM[1::2] = np.arange(32, 64)


def _to_dt(x: np.ndarray, dt: mybir.dt) -> np.ndarray:
    if dt == F32R:
        return round_fp32r(x)
    return np.ascontiguousarray(x).astype(mybir.dt.np(dt))


def _ktile_pack(a_t: np.ndarray) -> np.ndarray:
    """[HID, F] -> [128, KT*F] with k-tile-major free layout."""
    f = a_t.shape[1]
    return np.ascontiguousarray(
        a_t.reshape(KT, 128, f).transpose(1, 0, 2).reshape(128, KT * f)
    )


def prep_inputs(hidden_states, cos, sin, w_qkv, w_o):
    """Build the 8 per-core input maps."""
    hidden_states = np.asarray(hidden_states, dtype=np.float32)
    cos = np.asarray(cos, dtype=np.float32)
    sin = np.asarray(sin, dtype=np.float32)
    w_qkv = np.asarray(w_qkv, dtype=np.float32)
    w_o = np.asarray(w_o, dtype=np.float32)

    sgn = np.empty((64, 1), dtype=np.float32)
    sgn[0::2] = -1.0
    sgn[1::2] = 1.0
    cosx_half = cos.T[_PERM]                 # [64, T]
    sinx_half = sin.T[_PERM] * sgn           # [64, T]
    cosx = np.ascontiguousarray(np.concatenate([cosx_half, cosx_half], 0))
    sinx = np.ascontiguousarray(np.concatenate([sinx_half, sinx_half], 0))

    in_maps = []
    for c in range(NCORES):
        b, g = c // 4, c % 4
        r0 = g * 256
        wq_rows = w_qkv[r0: r0 + 256].reshape(4, 64, HID)[:, _PERM, :].reshape(256, HID)
        wk_rows = w_qkv[HID + r0: HID + r0 + 256].reshape(4, 64, HID)[:, _PERM, :]
        wk_rows = wk_rows.reshape(256, HID)
        wv_rows = w_qkv[2 * HID + r0: 2 * HID + r0 + 256]
        wo_cols = w_o[:, r0: r0 + 256]       # [HID, 256]

        in_maps.append({
            "hx": round_fp32r(_ktile_pack(hidden_states[b].T)),
            "wq": round_fp32r(_ktile_pack(wq_rows.T)),
            "wk": round_fp32r(_ktile_pack(wk_rows.T)),
            "wv": round_fp32r(_ktile_pack(wv_rows.T)),
            "wo": _to_dt(np.ascontiguousarray(
                wo_cols.T.reshape(2, 128, HID).transpose(1, 0, 2).reshape(128, 2 * HID)
            ), DT_O),
            "cosx": cosx,
            "sinx": sinx,
            "ones": np.ones((128, 64), dtype=ONES_NP),
        })
    return in_maps


def assemble_output(results):
    """Sum the 4 per-batch partials and transpose back to [B, T, HID]."""
    out = np.zeros((B, T, HID), dtype=np.float32)
    for c in range(NCORES):
        b = c // 4
        out[b] += results[c]["yt"].T
    return out


def run(inputs: dict, trace: bool = False, tmpdir: str | None = None):
    nc = build_nc()
    in_maps = prep_inputs(**inputs)
    res = bass_utils.run_bass_kernel_spmd(
        nc, in_maps, core_ids=list(range(NCORES)), trace=trace, tmpdir=tmpdir
    )
    return assemble_output(res.results), res


def kernel(**inputs) -> np.ndarray:
    out, _ = run(inputs, trace=False)
    return out


# revision 36
# speedup vs baseline: 1.1370x; 1.1370x over previous
"""Trainium2 Bass kernel for nn_Attention_42202348650800.

Full causal attention block: fused QKV projection + RoPE + causal softmax
attention + output projection.  B=2, T=2048, HIDDEN=1024, H=HKV=16, D=64.

Sharding (8 NeuronCores): data-parallel over batch (2) x tensor-parallel over
heads (4 groups of 4 heads).  core c -> batch b = c // 4, head group g = c % 4
(heads 4g..4g+3).  Each core computes a partial output projection
y_partial^T = w_o[:, jslice].T @ ctx^T  in [1024, 2048]; the host sums the 4
partials of each batch.

Device-side layout notes:
 - All matmul operands are float32r (fp32 with 11-bit mantissa; inputs
   pre-rounded on host).  PSUM accumulation is fp32.
 - Q^T/K^T are built directly in [d, t] layout (transposed), with the head-dim
   d PERMUTED per head as d' = (2i -> i, 2i+1 -> 32+i) so that RoPE's
   rotate-half becomes an even/odd partition swap (DVE stream_shuffle).
   cos/sin tables are pre-permuted and sign-folded on the host.
 - Scores are computed transposed (S^T[s, t]) so softmax's sum runs over the
   matmul contraction: ctx^T accumulates  V'^T @ expS^T  where V' = [V | 1];
   the ones column makes the softmax denominator fall out as row 64 of the
   ctx^T PSUM tile.
 - Causal mask is applied after exp by GPSIMD affine_select (fill 0).
"""

import math
import os
import sys

import numpy as np

sys.path.insert(0, "/opt/trn_rl_repo")

from contextlib import ExitStack

import concourse.bass as bass
import concourse.tile as tile
import concourse.mybir as mybir
from concourse import bacc, bass_utils

# Problem constants
B, T, HID = 2, 2048, 1024
H, D = 16, 64
NCORES = 8
HPC = 4          # heads per core
NPAIR = 2        # head pairs per core
KT = HID // 128  # 8 contraction tiles for qkv
F32 = mybir.dt.float32
F32R = mybir.dt.float32r
BF16 = mybir.dt.bfloat16
DT_E = BF16      # dtype of exp(scores) and V' (ctx matmul operands)
DT_QK = BF16     # dtype of Q^T/K^T tiles (scores matmul operands)
DT_O = BF16      # dtype of normalized ctx + w_o (output-proj operands)
ONES_NP = mybir.dt.np(DT_E)

SCALE = 1.0 / math.sqrt(D)

_NC_CACHE = {}


def round_fp32r(x: np.ndarray) -> np.ndarray:
    """Round fp32 to float32r grid (11-bit mantissa, RNE-ish)."""
    u = np.ascontiguousarray(x, dtype=np.float32).view(np.uint32).astype(np.uint64)
    u = (u + 0x7FF + ((u >> 12) & 1)) & ~np.uint64(0xFFF)
    return u.astype(np.uint32).view(np.float32)


def build_nc():
    """Build + compile the per-core Bass program (identical SPMD on all cores)."""
    if "nc" in _NC_CACHE:
        return _NC_CACHE["nc"]

    nc = bacc.Bacc("TRN2", target_bir_lowering=False, debug=False)

    # DRAM I/O (per core)
    hx_d = nc.dram_tensor("hx", [128, KT * T], F32R, kind="ExternalInput").ap()
    wq_d = nc.dram_tensor("wq", [128, KT * 256], F32R, kind="ExternalInput").ap()
    wk_d = nc.dram_tensor("wk", [128, KT * 256], F32R, kind="ExternalInput").ap()
    wv_d = nc.dram_tensor("wv", [128, KT * 256], F32R, kind="ExternalInput").ap()
    wo_d = nc.dram_tensor("wo", [128, 2 * HID], DT_O, kind="ExternalInput").ap()
    cos_d = nc.dram_tensor("cosx", [128, T], F32, kind="ExternalInput").ap()
    sin_d = nc.dram_tensor("sinx", [128, T], F32, kind="ExternalInput").ap()
    ones_d = nc.dram_tensor("ones", [128, 64], DT_E, kind="ExternalInput").ap()
    # causal-mask matmul constants: cols 0:128 = strict-upper -1e9 (lhsT),
    # cols 128:256 = identity (rhs)
    mski_d = nc.dram_tensor("mski", [128, 256], DT_QK, kind="ExternalInput").ap()
    yt_d = nc.dram_tensor("yt", [HID, T], F32, kind="ExternalOutput").ap()

    Exp = mybir.ActivationFunctionType.Exp
    is_ge = mybir.AluOpType.is_ge
    SWAP_MASK = [i ^ 1 for i in range(32)]

    with tile.TileContext(nc) as tc:
        with ExitStack() as octx:
            # ---- persistent pools (live for the whole kernel) ----
            qk_pool = octx.enter_context(tc.tile_pool(name="qk", bufs=1))
            vp_pool = octx.enter_context(tc.tile_pool(name="vp", bufs=1))

            QT = [qk_pool.tile([128, T], DT_QK, name=f"QT{p}") for p in range(NPAIR)]
            KTt = [qk_pool.tile([128, T], DT_QK, name=f"KTt{p}") for p in range(NPAIR)]
            Vp = vp_pool.tile([128, 16 * 260], DT_E, name="Vp")

            # ================= stage 1: QKV projection + RoPE + V' =========
            with ExitStack() as s1:
                in_pool = s1.enter_context(tc.tile_pool(name="inp", bufs=1))
                rp_pool = s1.enter_context(tc.tile_pool(name="ropep", bufs=1))
                ps1 = s1.enter_context(tc.tile_pool(name="ps1", bufs=1, space="PSUM"))

                wq_t = in_pool.tile([128, KT * 256], F32R, name="wq_t")
                wk_t = in_pool.tile([128, KT * 256], F32R, name="wk_t")
                cos_t = in_pool.tile([128, T], F32, name="cos_t")
                sin_t = in_pool.tile([128, T], F32, name="sin_t")
                hx_t = in_pool.tile([128, KT * T], F32R, name="hx_t")
                wv_t = in_pool.tile([128, KT * 256], F32R, name="wv_t")

                nc.sync.dma_start(wq_t[:], wq_d[:])
                nc.sync.dma_start(wk_t[:], wk_d[:])
                for k in range(KT):
                    nc.sync.dma_start(
                        hx_t[:, k * T:(k + 1) * T], hx_d[:, k * T:(k + 1) * T]
                    )
                nc.sync.dma_start(wv_t[:], wv_d[:])
                nc.sync.dma_start(cos_t[:], cos_d[:])
                nc.sync.dma_start(sin_t[:], sin_d[:])

                # ones columns of V' (softmax-denominator trick), DMA'd into
                # every 65th column
                vp_ones = Vp[:].rearrange("p (s h c) -> p s h c", s=16, h=4)
                nc.sync.dma_start(
                    vp_ones[:, :, :, 64:65],
                    ones_d[:].rearrange("p (s h c) -> p s h c", s=16, h=4),
                )

                # Q^T and K^T (pair-packed [2*64d, T]) + RoPE
                for pr in range(NPAIR):
                    for which, w_t, OUT in (("q", wq_t, QT), ("k", wk_t, KTt)):
                        raw = rp_pool.tile([128, T], F32, name=f"raw_{pr}{which}",
                                           tag="raw", bufs=2)
                        # k-outer: the first matmul only needs hx k-tile 0, so
                        # compute starts as soon as the first DMAs land
                        psqs = [
                            ps1.tile([128, 512], F32, name=f"psqk{pr}{which}{n}",
                                     tag="psqk", bufs=8)
                            for n in range(4)
                        ]
                        for k in range(KT):
                            for n in range(4):
                                nc.tensor.matmul(
                                    psqs[n][:],
                                    w_t[:, k * 256 + pr * 128: k * 256 + pr * 128 + 128],
                                    hx_t[:, k * T + n * 512: k * T + (n + 1) * 512],
                                    start=(k == 0), stop=(k == KT - 1),
                                )
                        for n in range(4):
                            nc.vector.tensor_copy(raw[:, n * 512:(n + 1) * 512],
                                                  psqs[n][:])
                        rot = rp_pool.tile([128, T], F32, name=f"rot_{pr}{which}",
                                           tag="rot")
                        nc.vector.stream_shuffle(rot[:], raw[:], SWAP_MASK)
                        tmp2 = rp_pool.tile([128, T], F32, name=f"ts_{pr}{which}",
                                            tag="tmp2")
                        nc.gpsimd.tensor_mul(tmp2[:], rot[:], sin_t[:])
                        # tmp1 reuses rot's slot (rot is dead once tmp2 is made)
                        tmp1 = rp_pool.tile([128, T], F32, name=f"tc_{pr}{which}",
                                            tag="rot")
                        nc.vector.tensor_mul(tmp1[:], raw[:], cos_t[:])
                        nc.vector.tensor_add(OUT[pr][:], tmp1[:], tmp2[:])

                # V' tiles: [V_h | 1] interleaved, natural d order
                for st in range(16):
                    psv = ps1.tile([128, 256], F32, name=f"psv{st}", tag="psqk",
                                   bufs=8)
                    for k in range(KT):
                        nc.tensor.matmul(
                            psv[:],
                            hx_t[:, k * T + st * 128: k * T + st * 128 + 128],
                            wv_t[:, k * 256:(k + 1) * 256],
                            start=(k == 0), stop=(k == KT - 1),
                        )
                    dst = Vp[:, st * 260:(st + 1) * 260]
                    dst = dst.rearrange("p (h c) -> p h c", h=4)[:, :, 0:64]
                    src = psv[:].rearrange("p (h c) -> p h c", h=4)
                    nc.vector.tensor_copy(dst, src)

            # late persistent pools (allocated after stage-1 pools release,
            # so they reuse stage-1's SBUF space)
            wo_pool = octx.enter_context(tc.tile_pool(name="wop", bufs=1))
            ctx_pool = octx.enter_context(tc.tile_pool(name="ctxp", bufs=1))
            dn_pool = octx.enter_context(tc.tile_pool(name="dnp", bufs=1))
            wo_t = wo_pool.tile([128, 2 * HID], DT_O, name="wo_t")
            nc.sync.dma_start(wo_t[:], wo_d[:])
            CTXU = [ctx_pool.tile([128, T], DT_O, name=f"CTXU{p}") for p in range(NPAIR)]
            mski_t = dn_pool.tile([128, 256], DT_QK, name="mski_t")
            nc.sync.dma_start(mski_t[:], mski_d[:])
            # one denominator row + reciprocal row per (pair, head, t-block):
            # compute-engine SBUF APs must start at partition 0/32/64/96, so
            # each row lives in its own [1, 512] tile.
            Dn = [dn_pool.tile([1, 512], F32, name=f"Dn{r}") for r in range(16)]
            Rn = [dn_pool.tile([1, 512], F32, name=f"Rn{r}") for r in range(16)]

            # ================= stage 2: attention ==========================
            with ExitStack() as s2:
                e_pool = s2.enter_context(tc.tile_pool(name="ep", bufs=1))
                ps_s = s2.enter_context(tc.tile_pool(name="pss", bufs=1, space="PSUM"))
                ps_c = s2.enter_context(tc.tile_pool(name="psc", bufs=1, space="PSUM"))

                NPIPE = 3  # software-pipeline depth: ctx matmuls trail the
                #            scores/exp stream so the PE FIFO never head-blocks
                for pr in range(NPAIR):
                    hA, hB = 2 * pr, 2 * pr + 1
                    for tb in range(4):
                        ctxP = [
                            ps_c.tile([65, 512], F32, name=f"ctx{pr}{tb}{ab}",
                                      tag="ctx", bufs=3)
                            for ab in range(2)
                        ]
                        shi = 4 * (tb + 1)
                        pend = []

                        def flush_ctx(pending):
                            si, col0, tiles = pending
                            for ab, hh in ((0, hA), (1, hB)):
                                nc.tensor.matmul(
                                    ctxP[ab][:, col0:],
                                    Vp[:, si * 260 + hh * 65: si * 260 + hh * 65 + 65],
                                    tiles[ab][:, col0:],
                                    start=(si == 0), stop=(si == shi - 1),
                                )

                        for si in range(shi):
                            j = si - 4 * tb
                            col0 = 0 if j < 0 else 128 * j
                            etiles = []
                            for ab, hh in ((0, hA), (1, hB)):
                                pp = slice(64 * ab, 64 * ab + 64)
                                pss = ps_s.tile([128, 512], F32,
                                                name=f"s{pr}{tb}{si}{ab}",
                                                tag="s", bufs=5)
                                nc.tensor.matmul(
                                    pss[:, col0:],
                                    KTt[pr][pp, si * 128:(si + 1) * 128],
                                    QT[pr][pp, tb * 512 + col0:(tb + 1) * 512],
                                    start=True, stop=(j < 0),
                                )
                                if j >= 0:
                                    # add the strict-lower causal mask (-1e9)
                                    # onto the 128-wide diagonal block, on PE
                                    nc.tensor.matmul(
                                        pss[:, col0:col0 + 128],
                                        mski_t[:, 0:128],
                                        mski_t[:, 128:256],
                                        start=False, stop=True,
                                        skip_group_check=True,
                                    )
                                ee = e_pool.tile([128, 512], DT_E,
                                                 name=f"e{pr}{tb}{si}{ab}",
                                                 tag="e", bufs=12)
                                nc.scalar.activation(ee[:, col0:], pss[:, col0:],
                                                     Exp, scale=SCALE)
                                etiles.append(ee)
                            pend.append((si, col0, etiles))
                            if len(pend) > NPIPE:
                                flush_ctx(pend.pop(0))
                        while pend:
                            flush_ctx(pend.pop(0))
                        # evacuate ctx + denominators
                        for ab, hh in ((0, hA), (1, hB)):
                            r = pr * 8 + ab * 4 + tb
                            nc.vector.tensor_copy(
                                CTXU[pr][64 * ab:64 * ab + 64, tb * 512:(tb + 1) * 512],
                                ctxP[ab][0:64, :],
                            )
                            nc.vector.tensor_copy(Dn[r][:], ctxP[ab][64:65, :])
                    # normalize this pair as soon as its denominators are done
                    for ab in range(2):
                        for tb in range(4):
                            r = pr * 8 + ab * 4 + tb
                            nc.vector.reciprocal_approx_fast(Rn[r][:], Dn[r][:])
                            # SBUF tensor_tensor requires equal base partitions,
                            # so broadcast to all 128 and slice the right half.
                            rb = e_pool.tile([128, 512], F32, name=f"rb{pr}{ab}{tb}",
                                             tag="rb", bufs=2)
                            nc.gpsimd.partition_broadcast(rb[:], Rn[r][:])
                            sl = slice(64 * ab, 64 * ab + 64)
                            cs = slice(tb * 512, (tb + 1) * 512)
                            nc.vector.tensor_mul(CTXU[pr][sl, cs], CTXU[pr][sl, cs],
                                                 rb[sl, :])

            # ================= stage 3: output projection ==================
            with ExitStack() as s3:
                y_pool = s3.enter_context(tc.tile_pool(name="yp", bufs=1))
                ps_y = s3.enter_context(tc.tile_pool(name="psy", bufs=1, space="PSUM"))
                for m in range(8):
                    yst = y_pool.tile([128, T], F32, name=f"yst{m}", tag="yst",
                                      bufs=2)
                    for n in range(4):
                        psy = ps_y.tile([128, 512], F32, name=f"psy{m}{n}", tag="psy",
                                        bufs=4)
                        for jt in range(2):
                            nc.tensor.matmul(
                                psy[:],
                                wo_t[:, jt * HID + m * 128: jt * HID + (m + 1) * 128],
                                CTXU[jt][:, n * 512:(n + 1) * 512],
                                start=(jt == 0), stop=(jt == 1),
                            )
                        if n % 2 == 0:
                            nc.vector.tensor_copy(yst[:, n * 512:(n + 1) * 512], psy[:])
                        else:
                            nc.scalar.copy(yst[:, n * 512:(n + 1) * 512], psy[:])
                    nc.sync.dma_start(yt_d[m * 128:(m + 1) * 128, :], yst[:])

    nc.compile()
    _NC_CACHE["nc"] = nc
    return nc


# RoPE head-dim permutation: d' = 2i -> i, 2i+1 -> 32+i
_PERM = np.empty(64, dtype=np.int64)
_PERM[0::2] = np.arange(32)
_PERM[1::2] = np.arange(32, 64)


def _mski() -> np.ndarray:
    maskT = np.triu(np.full((128, 128), -1e9, dtype=np.float32), 1)
    ident = np.eye(128, dtype=np.float32)
    return np.concatenate([maskT, ident], 1).astype(mybir.dt.np(DT_QK))


def _to_dt(x: np.ndarray, dt: mybir.dt) -> np.ndarray:
    if dt == F32R:
        return round_fp32r(x)
    return np.ascontiguousarray(x).astype(mybir.dt.np(dt))


def _ktile_pack(a_t: np.ndarray) -> np.ndarray:
    """[HID, F] -> [128, KT*F] with k-tile-major free layout."""
    f = a_t.shape[1]
    return np.ascontiguousarray(
        a_t.reshape(KT, 128, f).transpose(1, 0, 2).reshape(128, KT * f)
    )


def prep_inputs(hidden_states, cos, sin, w_qkv, w_o):
    """Build the 8 per-core input maps."""
    hidden_states = np.asarray(hidden_states, dtype=np.float32)
    cos = np.asarray(cos, dtype=np.float32)
    sin = np.asarray(sin, dtype=np.float32)
    w_qkv = np.asarray(w_qkv, dtype=np.float32)
    w_o = np.asarray(w_o, dtype=np.float32)

    sgn = np.empty((64, 1), dtype=np.float32)
    sgn[0::2] = -1.0
    sgn[1::2] = 1.0
    cosx_half = cos.T[_PERM]                 # [64, T]
    sinx_half = sin.T[_PERM] * sgn           # [64, T]
    cosx = np.ascontiguousarray(np.concatenate([cosx_half, cosx_half], 0))
    sinx = np.ascontiguousarray(np.concatenate([sinx_half, sinx_half], 0))

    in_maps = []
    for c in range(NCORES):
        b, g = c // 4, c % 4
        r0 = g * 256
        wq_rows = w_qkv[r0: r0 + 256].reshape(4, 64, HID)[:, _PERM, :].reshape(256, HID)
        wk_rows = w_qkv[HID + r0: HID + r0 + 256].reshape(4, 64, HID)[:, _PERM, :]
        wk_rows = wk_rows.reshape(256, HID)
        wv_rows = w_qkv[2 * HID + r0: 2 * HID + r0 + 256]
        wo_cols = w_o[:, r0: r0 + 256]       # [HID, 256]

        in_maps.append({
            "hx": round_fp32r(_ktile_pack(hidden_states[b].T)),
            "wq": round_fp32r(_ktile_pack(wq_rows.T)),
            "wk": round_fp32r(_ktile_pack(wk_rows.T)),
            "wv": round_fp32r(_ktile_pack(wv_rows.T)),
            "wo": _to_dt(np.ascontiguousarray(
                wo_cols.T.reshape(2, 128, HID).transpose(1, 0, 2).reshape(128, 2 * HID)
            ), DT_O),
            "cosx": cosx,
            "sinx": sinx,
            "ones": np.ones((128, 64), dtype=ONES_NP),
            "mski": _mski(),
        })
    return in_maps


def assemble_output(results):
    """Sum the 4 per-batch partials and transpose back to [B, T, HID]."""
    out = np.zeros((B, T, HID), dtype=np.float32)
    for c in range(NCORES):
        b = c // 4
        out[b] += results[c]["yt"].T
    return out


def run(inputs: dict, trace: bool = False, tmpdir: str | None = None):
    nc = build_nc()
    in_maps = prep_inputs(**inputs)
    res = bass_utils.run_bass_kernel_spmd(
        nc, in_maps, core_ids=list(range(NCORES)), trace=trace, tmpdir=tmpdir
    )
    return assemble_output(res.results), res


def kernel(**inputs) -> np.ndarray:
    out, _ = run(inputs, trace=False)
    return out


# revision 43
# speedup vs baseline: 1.3076x; 1.1500x over previous
"""Trainium2 Bass kernel for nn_Attention_42202348650800.

Full causal attention block: fused QKV projection + RoPE + causal softmax
attention + output projection.  B=2, T=2048, HIDDEN=1024, H=HKV=16, D=64.

Sharding (8 NeuronCores): data-parallel over batch (2) x tensor-parallel over
heads (4 groups of 4 heads).  core c -> batch b = c // 4, head group g = c % 4
(heads 4g..4g+3).  Each core computes a partial output projection
y_partial^T = w_o[:, jslice].T @ ctx^T  in [1024, 2048]; the host sums the 4
partials of each batch.

Device-side layout notes:
 - All matmul operands are float32r (fp32 with 11-bit mantissa; inputs
   pre-rounded on host).  PSUM accumulation is fp32.
 - Q^T/K^T are built directly in [d, t] layout (transposed), with the head-dim
   d PERMUTED per head as d' = (2i -> i, 2i+1 -> 32+i) so that RoPE's
   rotate-half becomes an even/odd partition swap (DVE stream_shuffle).
   cos/sin tables are pre-permuted and sign-folded on the host.
 - Scores are computed transposed (S^T[s, t]) so softmax's sum runs over the
   matmul contraction: ctx^T accumulates  V'^T @ expS^T  where V' = [V | 1];
   the ones column makes the softmax denominator fall out as row 64 of the
   ctx^T PSUM tile.
 - Causal mask is applied after exp by GPSIMD affine_select (fill 0).
"""

import math
import os
import sys

import numpy as np

sys.path.insert(0, "/opt/trn_rl_repo")

from contextlib import ExitStack

import concourse.bass as bass
import concourse.tile as tile
import concourse.mybir as mybir
from concourse import bacc, bass_utils

# Problem constants
B, T, HID = 2, 2048, 1024
H, D = 16, 64
NCORES = 8
HPC = 4          # heads per core
NPAIR = 2        # head pairs per core
KT = HID // 128  # 8 contraction tiles for qkv
F32 = mybir.dt.float32
F32R = mybir.dt.float32r
BF16 = mybir.dt.bfloat16
DT_E = BF16      # dtype of exp(scores) and V' (ctx matmul operands)
DT_QK = BF16     # dtype of Q^T/K^T tiles (scores matmul operands)
DT_O = BF16      # dtype of normalized ctx + w_o (output-proj operands)
DT_X = BF16      # dtype of hidden_states / qkv weights (QKV matmul operands)
ONES_NP = mybir.dt.np(DT_E)

SCALE = 1.0 / math.sqrt(D)

_NC_CACHE = {}


def round_fp32r(x: np.ndarray) -> np.ndarray:
    """Round fp32 to float32r grid (11-bit mantissa, RNE-ish)."""
    u = np.ascontiguousarray(x, dtype=np.float32).view(np.uint32).astype(np.uint64)
    u = (u + 0x7FF + ((u >> 12) & 1)) & ~np.uint64(0xFFF)
    return u.astype(np.uint32).view(np.float32)


def build_nc():
    """Build + compile the per-core Bass program (identical SPMD on all cores)."""
    if "nc" in _NC_CACHE:
        return _NC_CACHE["nc"]

    nc = bacc.Bacc("TRN2", target_bir_lowering=False, debug=False)

    # DRAM I/O (per core)
    hx_d = nc.dram_tensor("hx", [128, KT * T], DT_X, kind="ExternalInput").ap()
    wq_d = nc.dram_tensor("wq", [128, KT * 256], DT_X, kind="ExternalInput").ap()
    wk_d = nc.dram_tensor("wk", [128, KT * 256], DT_X, kind="ExternalInput").ap()
    wv_d = nc.dram_tensor("wv", [128, KT * 256], DT_X, kind="ExternalInput").ap()
    wo_d = nc.dram_tensor("wo", [128, 2 * HID], DT_O, kind="ExternalInput").ap()
    cos_d = nc.dram_tensor("cosx", [128, T], F32, kind="ExternalInput").ap()
    sin_d = nc.dram_tensor("sinx", [128, T], F32, kind="ExternalInput").ap()
    ones_d = nc.dram_tensor("ones", [128, 64], DT_E, kind="ExternalInput").ap()
    # causal-mask matmul constants: cols 0:128 = strict-upper -1e9 (lhsT),
    # cols 128:256 = identity (rhs)
    mski_d = nc.dram_tensor("mski", [128, 256], DT_QK, kind="ExternalInput").ap()
    yt_d = nc.dram_tensor("yt", [HID, T], F32, kind="ExternalOutput").ap()

    Exp = mybir.ActivationFunctionType.Exp
    is_ge = mybir.AluOpType.is_ge
    SWAP_MASK = [i ^ 1 for i in range(32)]

    with tile.TileContext(nc) as tc:
        with ExitStack() as octx:
            # ---- persistent pools (live for the whole kernel) ----
            qk_pool = octx.enter_context(tc.tile_pool(name="qk", bufs=1))
            vp_pool = octx.enter_context(tc.tile_pool(name="vp", bufs=1))

            QT = [qk_pool.tile([128, T], DT_QK, name=f"QT{p}") for p in range(NPAIR)]
            KTt = [qk_pool.tile([128, T], DT_QK, name=f"KTt{p}") for p in range(NPAIR)]
            Vp = vp_pool.tile([128, 16 * 260], DT_E, name="Vp")

            # ================= stage 1: QKV projection + RoPE + V' =========
            with ExitStack() as s1:
                in_pool = s1.enter_context(tc.tile_pool(name="inp", bufs=1))
                rp_pool = s1.enter_context(tc.tile_pool(name="ropep", bufs=1))
                ps1 = s1.enter_context(tc.tile_pool(name="ps1", bufs=1, space="PSUM"))

                wq_t = in_pool.tile([128, KT * 256], DT_X, name="wq_t")
                wk_t = in_pool.tile([128, KT * 256], DT_X, name="wk_t")
                cos_t = in_pool.tile([128, T], F32, name="cos_t")
                sin_t = in_pool.tile([128, T], F32, name="sin_t")
                hx_t = in_pool.tile([128, KT * T], DT_X, name="hx_t")
                wv_t = in_pool.tile([128, KT * 256], DT_X, name="wv_t")

                nc.sync.dma_start(wq_t[:], wq_d[:])
                for k in range(2):
                    nc.sync.dma_start(
                        hx_t[:, k * T:(k + 1) * T], hx_d[:, k * T:(k + 1) * T]
                    )
                nc.sync.dma_start(wk_t[:], wk_d[:])
                for k in range(2, KT):
                    nc.sync.dma_start(
                        hx_t[:, k * T:(k + 1) * T], hx_d[:, k * T:(k + 1) * T]
                    )
                nc.sync.dma_start(wv_t[:], wv_d[:])
                nc.sync.dma_start(cos_t[:], cos_d[:])
                nc.sync.dma_start(sin_t[:], sin_d[:])

                # ones columns of V' (softmax-denominator trick), DMA'd into
                # every 65th column
                vp_ones = Vp[:].rearrange("p (s h c) -> p s h c", s=16, h=4)
                nc.sync.dma_start(
                    vp_ones[:, :, :, 64:65],
                    ones_d[:].rearrange("p (s h c) -> p s h c", s=16, h=4),
                )

                # Q^T and K^T (pair-packed [2*64d, T]) + RoPE
                for pr in range(NPAIR):
                    for which, w_t, OUT in (("q", wq_t, QT), ("k", wk_t, KTt)):
                        raw = rp_pool.tile([128, T], F32, name=f"raw_{pr}{which}",
                                           tag="raw", bufs=2)
                        # k-outer: the first matmul only needs hx k-tile 0, so
                        # compute starts as soon as the first DMAs land
                        psqs = [
                            ps1.tile([128, 512], F32, name=f"psqk{pr}{which}{n}",
                                     tag="psqk", bufs=8)
                            for n in range(4)
                        ]
                        for k in range(KT):
                            for n in range(4):
                                nc.tensor.matmul(
                                    psqs[n][:],
                                    w_t[:, k * 256 + pr * 128: k * 256 + pr * 128 + 128],
                                    hx_t[:, k * T + n * 512: k * T + (n + 1) * 512],
                                    start=(k == 0), stop=(k == KT - 1),
                                )
                        for n in range(4):
                            nc.vector.tensor_copy(raw[:, n * 512:(n + 1) * 512],
                                                  psqs[n][:])
                        rot = rp_pool.tile([128, T], F32, name=f"rot_{pr}{which}",
                                           tag="rot")
                        nc.vector.stream_shuffle(rot[:], raw[:], SWAP_MASK)
                        tmp2 = rp_pool.tile([128, T], F32, name=f"ts_{pr}{which}",
                                            tag="tmp2")
                        nc.gpsimd.tensor_mul(tmp2[:], rot[:], sin_t[:])
                        # tmp1 reuses rot's slot (rot is dead once tmp2 is made)
                        tmp1 = rp_pool.tile([128, T], F32, name=f"tc_{pr}{which}",
                                            tag="rot")
                        nc.vector.tensor_mul(tmp1[:], raw[:], cos_t[:])
                        nc.vector.tensor_add(OUT[pr][:], tmp1[:], tmp2[:])

                # V' tiles: [V_h | 1] interleaved, natural d order
                for st in range(16):
                    psv = ps1.tile([128, 256], F32, name=f"psv{st}", tag="psqk",
                                   bufs=8)
                    for k in range(KT):
                        nc.tensor.matmul(
                            psv[:],
                            hx_t[:, k * T + st * 128: k * T + st * 128 + 128],
                            wv_t[:, k * 256:(k + 1) * 256],
                            start=(k == 0), stop=(k == KT - 1),
                        )
                    dst = Vp[:, st * 260:(st + 1) * 260]
                    dst = dst.rearrange("p (h c) -> p h c", h=4)[:, :, 0:64]
                    src = psv[:].rearrange("p (h c) -> p h c", h=4)
                    nc.vector.tensor_copy(dst, src)

            # late persistent pools (allocated after stage-1 pools release,
            # so they reuse stage-1's SBUF space)
            wo_pool = octx.enter_context(tc.tile_pool(name="wop", bufs=1))
            ctx_pool = octx.enter_context(tc.tile_pool(name="ctxp", bufs=1))
            dn_pool = octx.enter_context(tc.tile_pool(name="dnp", bufs=1))
            wo_t = wo_pool.tile([128, 2 * HID], DT_O, name="wo_t")
            nc.sync.dma_start(wo_t[:], wo_d[:])
            CTXU = [ctx_pool.tile([128, T], DT_O, name=f"CTXU{p}") for p in range(NPAIR)]
            mski_t = dn_pool.tile([128, 256], DT_QK, name="mski_t")
            nc.sync.dma_start(mski_t[:], mski_d[:])
            # denominator rows, all on partition 0 of one tile (compute-engine
            # SBUF APs must start at partition 0/32/64/96)
            Dn = dn_pool.tile([1, 16 * 512], F32, name="Dn")

            # ================= stage 2: attention ==========================
            with ExitStack() as s2:
                e_pool = s2.enter_context(tc.tile_pool(name="ep", bufs=1))
                ps_s = s2.enter_context(tc.tile_pool(name="pss", bufs=1, space="PSUM"))
                ps_c = s2.enter_context(tc.tile_pool(name="psc", bufs=1, space="PSUM"))

                NPIPE = 3  # software-pipeline depth: ctx matmuls trail the
                #            scores/exp stream so the PE FIFO never head-blocks
                for pr in range(NPAIR):
                    hA, hB = 2 * pr, 2 * pr + 1
                    for tb in range(4):
                        ctxP = [
                            ps_c.tile([65, 512], F32, name=f"ctx{pr}{tb}{ab}",
                                      tag="ctx", bufs=3)
                            for ab in range(2)
                        ]
                        shi = 4 * (tb + 1)
                        pend = []

                        def flush_ctx(pending):
                            si, col0, tiles = pending
                            for ab, hh in ((0, hA), (1, hB)):
                                nc.tensor.matmul(
                                    ctxP[ab][:, col0:],
                                    Vp[:, si * 260 + hh * 65: si * 260 + hh * 65 + 65],
                                    tiles[ab][:, col0:],
                                    start=(si == 0), stop=(si == shi - 1),
                                )

                        for si in range(shi):
                            j = si - 4 * tb
                            col0 = 0 if j < 0 else 128 * j
                            etiles = []
                            for ab, hh in ((0, hA), (1, hB)):
                                pp = slice(64 * ab, 64 * ab + 64)
                                pss = ps_s.tile([128, 512], F32,
                                                name=f"s{pr}{tb}{si}{ab}",
                                                tag="s", bufs=5)
                                nc.tensor.matmul(
                                    pss[:, col0:],
                                    KTt[pr][pp, si * 128:(si + 1) * 128],
                                    QT[pr][pp, tb * 512 + col0:(tb + 1) * 512],
                                    start=True, stop=(j < 0),
                                )
                                if j >= 0:
                                    # add the strict-lower causal mask (-1e9)
                                    # onto the 128-wide diagonal block, on PE
                                    nc.tensor.matmul(
                                        pss[:, col0:col0 + 128],
                                        mski_t[:, 0:128],
                                        mski_t[:, 128:256],
                                        start=False, stop=True,
                                        skip_group_check=True,
                                    )
                                ee = e_pool.tile([128, 512], DT_E,
                                                 name=f"e{pr}{tb}{si}{ab}",
                                                 tag="e", bufs=12)
                                nc.scalar.activation(ee[:, col0:], pss[:, col0:],
                                                     Exp, scale=SCALE)
                                etiles.append(ee)
                            pend.append((si, col0, etiles))
                            if len(pend) > NPIPE:
                                flush_ctx(pend.pop(0))
                        while pend:
                            flush_ctx(pend.pop(0))
                        # evacuate ctx + denominators
                        for ab, hh in ((0, hA), (1, hB)):
                            r = pr * 8 + ab * 4 + tb
                            nc.vector.tensor_copy(
                                CTXU[pr][64 * ab:64 * ab + 64, tb * 512:(tb + 1) * 512],
                                ctxP[ab][0:64, :],
                            )
                            nc.vector.tensor_copy(
                                Dn[0:1, r * 512:(r + 1) * 512], ctxP[ab][64:65, :]
                            )
                    # normalize this pair as soon as its denominators are done
                    for ab in range(2):
                        for tb in range(4):
                            r = pr * 8 + ab * 4 + tb
                            # broadcast the denominator row to all partitions,
                            # take the reciprocal in place, multiply.
                            rb = e_pool.tile([128, 512], F32, name=f"rb{pr}{ab}{tb}",
                                             tag="rb", bufs=2)
                            nc.gpsimd.partition_broadcast(
                                rb[:], Dn[0:1, r * 512:(r + 1) * 512]
                            )
                            nc.vector.reciprocal_approx_fast(rb[:], rb[:])
                            sl = slice(64 * ab, 64 * ab + 64)
                            cs = slice(tb * 512, (tb + 1) * 512)
                            nc.vector.tensor_mul(CTXU[pr][sl, cs], CTXU[pr][sl, cs],
                                                 rb[sl, :])

            # ================= stage 3: output projection ==================
            with ExitStack() as s3:
                y_pool = s3.enter_context(tc.tile_pool(name="yp", bufs=1))
                ps_y = s3.enter_context(tc.tile_pool(name="psy", bufs=1, space="PSUM"))
                for m in range(8):
                    yst = y_pool.tile([128, T], F32, name=f"yst{m}", tag="yst",
                                      bufs=2)
                    for n in range(4):
                        psy = ps_y.tile([128, 512], F32, name=f"psy{m}{n}", tag="psy",
                                        bufs=4)
                        for jt in range(2):
                            nc.tensor.matmul(
                                psy[:],
                                wo_t[:, jt * HID + m * 128: jt * HID + (m + 1) * 128],
                                CTXU[jt][:, n * 512:(n + 1) * 512],
                                start=(jt == 0), stop=(jt == 1),
                            )
                        if n % 2 == 0:
                            nc.vector.tensor_copy(yst[:, n * 512:(n + 1) * 512], psy[:])
                        else:
                            nc.scalar.copy(yst[:, n * 512:(n + 1) * 512], psy[:])
                    nc.sync.dma_start(yt_d[m * 128:(m + 1) * 128, :], yst[:])

    nc.compile()
    _NC_CACHE["nc"] = nc
    return nc


# RoPE head-dim permutation: d' = 2i -> i, 2i+1 -> 32+i
_PERM = np.empty(64, dtype=np.int64)
_PERM[0::2] = np.arange(32)
_PERM[1::2] = np.arange(32, 64)


def _mski() -> np.ndarray:
    maskT = np.triu(np.full((128, 128), -1e9, dtype=np.float32), 1)
    ident = np.eye(128, dtype=np.float32)
    return np.concatenate([maskT, ident], 1).astype(mybir.dt.np(DT_QK))


def _to_dt(x: np.ndarray, dt: mybir.dt) -> np.ndarray:
    if dt == F32R:
        return round_fp32r(x)
    return np.ascontiguousarray(x).astype(mybir.dt.np(dt))


def _ktile_pack(a_t: np.ndarray) -> np.ndarray:
    """[HID, F] -> [128, KT*F] with k-tile-major free layout."""
    f = a_t.shape[1]
    return np.ascontiguousarray(
        a_t.reshape(KT, 128, f).transpose(1, 0, 2).reshape(128, KT * f)
    )


def prep_inputs(hidden_states, cos, sin, w_qkv, w_o):
    """Build the 8 per-core input maps."""
    hidden_states = np.asarray(hidden_states, dtype=np.float32)
    cos = np.asarray(cos, dtype=np.float32)
    sin = np.asarray(sin, dtype=np.float32)
    w_qkv = np.asarray(w_qkv, dtype=np.float32)
    w_o = np.asarray(w_o, dtype=np.float32)

    sgn = np.empty((64, 1), dtype=np.float32)
    sgn[0::2] = -1.0
    sgn[1::2] = 1.0
    cosx_half = cos.T[_PERM]                 # [64, T]
    sinx_half = sin.T[_PERM] * sgn           # [64, T]
    cosx = np.ascontiguousarray(np.concatenate([cosx_half, cosx_half], 0))
    sinx = np.ascontiguousarray(np.concatenate([sinx_half, sinx_half], 0))

    in_maps = []
    for c in range(NCORES):
        b, g = c // 4, c % 4
        r0 = g * 256
        wq_rows = w_qkv[r0: r0 + 256].reshape(4, 64, HID)[:, _PERM, :].reshape(256, HID)
        wk_rows = w_qkv[HID + r0: HID + r0 + 256].reshape(4, 64, HID)[:, _PERM, :]
        wk_rows = wk_rows.reshape(256, HID)
        wv_rows = w_qkv[2 * HID + r0: 2 * HID + r0 + 256]
        wo_cols = w_o[:, r0: r0 + 256]       # [HID, 256]

        in_maps.append({
            "hx": _to_dt(_ktile_pack(hidden_states[b].T), DT_X),
            "wq": _to_dt(_ktile_pack(wq_rows.T), DT_X),
            "wk": _to_dt(_ktile_pack(wk_rows.T), DT_X),
            "wv": _to_dt(_ktile_pack(wv_rows.T), DT_X),
            "wo": _to_dt(np.ascontiguousarray(
                wo_cols.T.reshape(2, 128, HID).transpose(1, 0, 2).reshape(128, 2 * HID)
            ), DT_O),
            "cosx": cosx,
            "sinx": sinx,
            "ones": np.ones((128, 64), dtype=ONES_NP),
            "mski": _mski(),
        })
    return in_maps


def assemble_output(results):
    """Sum the 4 per-batch partials and transpose back to [B, T, HID]."""
    out = np.zeros((B, T, HID), dtype=np.float32)
    for c in range(NCORES):
        b = c // 4
        out[b] += results[c]["yt"].T
    return out


def run(inputs: dict, trace: bool = False, tmpdir: str | None = None):
    nc = build_nc()
    in_maps = prep_inputs(**inputs)
    res = bass_utils.run_bass_kernel_spmd(
        nc, in_maps, core_ids=list(range(NCORES)), trace=trace, tmpdir=tmpdir
    )
    return assemble_output(res.results), res


def kernel(**inputs) -> np.ndarray:
    out, _ = run(inputs, trace=False)
    return out
